# revision 1
# baseline (speedup 1.0000x reference)
"""Trainium2 8-core kernel for nn_Block_47794396070541 (attention + top-2 MoE +
shared MLP transformer block).

Strategy (full inputs in, full output out; sharded internally over 8 cores):

Launch A (attention, tensor-parallel over heads):
  Each core owns 2 of 16 q-heads (and their shared kv head) for both batches,
  computes qkv projection, qk-norm, partial rope, causal attention with the
  softmax denominator obtained via an appended ones-column on V, applies the
  sigmoid gate, and emits a partial product against its 128 rows of w_o.
  The host sums the 8 partials (the all-reduce) and forms h = x + attn.

Host (cheap numpy): rms norms, router softmax + top-2, token dispatch.

Launch B (MoE expert-parallel + shared MLP token-parallel):
  Core e receives the tokens routed to expert e (gathered, padded to C),
  runs silu(x@Wg)*(x@Wu) @ Wd scaled by the combine weight, plus the shared
  MLP for 1/8 of the tokens. Host scatter-adds expert outputs and assembles
  out = h + moe + shared.

Everything matmul-shaped runs on the TensorEngine in bf16 with f32
accumulation; softmax exp runs on the ScalarEngine (scores are bounded by
+-8 after qk-norm so no max-subtraction is needed).
"""

from contextlib import ExitStack

import numpy as np
import ml_dtypes

import concourse.mybir as mybir
import concourse.tile as tile
from concourse import bacc
from concourse.bass_utils import run_bass_kernel_spmd
from concourse.masks import make_identity

F32 = mybir.dt.float32
BF16 = mybir.dt.bfloat16
AF = mybir.ActivationFunctionType

# problem shapes
B, S, D = 2, 2048, 1024
T = B * S
NH, MH, HD = 16, 4, 64
G = 12
E, K, I = 8, 2, 1024
ISH = 1024
EPS = 1e-5
QK_EPS = 1e-6
ROPE_THETA = 1024.0
ROT_DIM = 32
P = 128
NB = B
SC = S // P
N_CORES = 8

_cache = {}


def _bf16(a):
    return np.asarray(a).astype(ml_dtypes.bfloat16)


# --------------------------------------------------------------------------
# Launch A builder: attention (2 q-heads per core)
# --------------------------------------------------------------------------
def _build_attn():
    nc = bacc.Bacc(None, target_bir_lowering=False, debug=False)

    xT = nc.declare_dram_parameter("xT", [D, T], BF16, isOutput=False)
    wpack = nc.declare_dram_parameter("wpack", [D, 256], BF16, isOutput=False)
    wo = nc.declare_dram_parameter("wo", [P, D], BF16, isOutput=False)
    gateT = nc.declare_dram_parameter("gateT", [2, T], F32, isOutput=False)
    cos3 = nc.declare_dram_parameter("cos3", [S, 48], F32, isOutput=False)
    sin3 = nc.declare_dram_parameter("sin3", [S, 48], F32, isOutput=False)
    mask = nc.declare_dram_parameter("mask", [P, 1024], BF16, isOutput=False)
    po = nc.declare_dram_parameter("po", [T, D], F32, isOutput=True)

    with tile.TileContext(nc) as tc, ExitStack() as ctx:
        const = ctx.enter_context(tc.tile_pool(name="const", bufs=1))
        work = ctx.enter_context(tc.tile_pool(name="work", bufs=4))
        exps = ctx.enter_context(tc.tile_pool(name="exps", bufs=8))

        # DMA issue order matters: the first projection needs wp + xT's
        # first t-quarter, so those descriptors go out before everything else
        wp_sb = const.tile([P, 8, 256], BF16)
        nc.sync.dma_start(wp_sb[:], wpack.rearrange("(a p) c -> p a c", p=P))
        xT_sb = const.tile([P, 8, T], BF16)
        xT_r = xT.rearrange("(a p) c -> p a c", p=P)
        tsl0 = slice(0, T // 4)
        for d in range(8):
            (nc.sync if d % 2 == 0 else nc.gpsimd).dma_start(
                xT_sb[:, d, tsl0], xT_r[:, d, tsl0])
        wo_sb = const.tile([P, D], BF16)
        nc.gpsimd.dma_start(wo_sb[:], wo[:])
        cos_sb = const.tile([P, SC, 3, 16], F32)
        nc.sync.dma_start(cos_sb[:], cos3.rearrange("(a p) (g j) -> p a g j", p=P, g=3))
        sin_sb = const.tile([P, SC, 3, 16], F32)
        nc.sync.dma_start(sin_sb[:], sin3.rearrange("(a p) (g j) -> p a g j", p=P, g=3))
        mask_sb = const.tile([P, 1024], BF16)
        nc.sync.dma_start(mask_sb[:], mask[:])
        ident = const.tile([P, P], F32)
        make_identity(nc, ident[:])
        ones_sb = const.tile([1, HD], F32)
        nc.vector.memset(ones_sb[:], 1.0)
        eps_sb = const.tile([P, 1], F32)
        nc.vector.memset(eps_sb[:], 1e-6)

        dma_engines = [nc.sync, nc.gpsimd]
        for tq in range(1, 4):
            tsl = slice(tq * (T // 4), (tq + 1) * (T // 4))
            for d in range(8):
                dma_engines[d % 2].dma_start(xT_sb[:, d, tsl], xT_r[:, d, tsl])

        # packed transposed layouts: rows 0-63 = head 0, rows 64-127 = head 1
        # (kT is the shared kv head duplicated into both halves)
        qT_sb = [const.tile([P, S], BF16, tag=f"qT{b}", name=f"qT{b}")
                 for b in range(NB)]
        kT_sb = [const.tile([P, S], BF16, tag=f"kT{b}", name=f"kT{b}")
                 for b in range(NB)]
        v_sb = [const.tile([P, SC, HD + 1], BF16, tag=f"v{b}", name=f"v{b}")
                for b in range(NB)]

        ph1_cm = tc.tile_pool(name="ph1", bufs=1, space="PSUM")
        ph1 = ph1_cm.__enter__()
        for b in range(NB):
            nc.vector.memset(v_sb[b][:, :, HD:HD + 1], 1.0)
        SB = 2  # s-chunks batched per iteration (op-count reduction)
        for b in range(NB):
            for sc2 in range(SC // SB):
                sc0 = sc2 * SB
                t0 = b * S + sc0 * P
                pp = ph1.tile([P, SB, 256], F32, tag="proj", bufs=4,
                              name=f"pp{b}_{sc2}")
                for j in range(SB):
                    for d in range(8):
                        nc.tensor.matmul(pp[:, j], xT_sb[:, d, t0 + j * P:
                                                        t0 + (j + 1) * P],
                                         wp_sb[:, d, :],
                                         start=(d == 0), stop=(d == 7))
                sq = work.tile([P, SB, 3, HD], F32, tag="sq", bufs=4)
                nc.scalar.activation(sq[:], pp[:, :, 0:192], AF.Square)
                ssum = work.tile([P, SB, 3], F32, tag="ssum", bufs=4)
                nc.vector.reduce_sum(ssum[:], sq[:], axis=mybir.AxisListType.X)
                rstd = work.tile([P, SB, 3, 1], F32, tag="rstd", bufs=4)
                nc.scalar.activation(rstd[:], ssum[:], AF.Sqrt,
                                     scale=1.0 / HD, bias=eps_sb[:])
                nc.vector.reciprocal(rstd[:], rstd[:])
                qkv = work.tile([P, SB, 3, HD], F32, tag="qkv", bufs=4)
                nc.vector.tensor_mul(
                    qkv[:], pp[:, :, 0:192].rearrange("p a (g d) -> p a g d", g=3),
                    rstd[:].to_broadcast((P, SB, 3, HD)))
                x1 = qkv[:, :, :, 0:16]
                x2 = qkv[:, :, :, 16:32]
                cs = cos_sb[:, sc0:sc0 + SB]
                sn = sin_sb[:, sc0:sc0 + SB]
                tmp = work.tile([P, 4, SB, 3, 16], F32, tag="ropetmp", bufs=4)
                nc.vector.tensor_mul(tmp[:, 0], x1, cs)
                nc.vector.tensor_mul(tmp[:, 1], x2, sn)
                nc.vector.tensor_mul(tmp[:, 2], x2, cs)
                nc.vector.tensor_mul(tmp[:, 3], x1, sn)
                nc.vector.tensor_sub(x1, tmp[:, 0], tmp[:, 1])
                nc.vector.tensor_add(x2, tmp[:, 2], tmp[:, 3])
                nc.scalar.copy(v_sb[b][:, sc0:sc0 + SB, 0:HD], pp[:, :, 192:256])
                for j in range(SB):
                    sc = sc0 + j
                    tq = ph1.tile([P, P], F32, tag="tr", bufs=4,
                                  name=f"tq{b}_{sc}")
                    nc.tensor.transpose(tq[:], qkv[:, j, 0:2, :], ident[:])
                    nc.scalar.copy(qT_sb[b][:, sc * P:(sc + 1) * P], tq[:])
                    tk = ph1.tile([HD, P], F32, tag="tr", bufs=4,
                                  name=f"tk{b}_{sc}")
                    nc.tensor.transpose(tk[:], qkv[:, j, 2, :], ident[:])
                    nc.scalar.copy(kT_sb[b][0:HD, sc * P:(sc + 1) * P], tk[:])
            # duplicate the kv head into rows 64-127 (head-1 half) via
            # SBUF->SBUF DMA; engines cannot shift partitions but DMA can
            nc.gpsimd.dma_start(kT_sb[b][HD:P, :], kT_sb[b][0:HD, :])

        ph1_cm.__exit__(None, None, None)  # release phase-1 banks
        # phase 2: attention + w_o partial (both heads interleaved so PE never
        # waits on the per-chunk exp)
        ps = ctx.enter_context(tc.tile_pool(name="ps", bufs=1, space="PSUM"))
        QT = 512
        for b in range(NB):
            for qt in range(S // QT):
                attnT2 = work.tile([P, QT], BF16, tag="attnT2",
                                   name=f"attnT2_{b}_{qt}")
                op = [ps.tile([HD + 1, QT], F32, tag=f"outp{h}",
                              name=f"op{b}_{qt}_{h}") for h in range(2)]
                nkv = 4 * qt + 4

                def emit_out(c, ex2):
                    qlo = max(0, c * P - qt * QT)
                    for h in range(2):
                        nc.tensor.matmul(op[h][:, qlo:], v_sb[b][:, c, :],
                                         ex2[:, h, qlo:],
                                         start=(c == 0), stop=(c == nkv - 1))

                # software-pipelined by 2 chunks: the out matmuls for chunk c
                # are emitted after the scores/exp of chunk c+2, so the PE
                # always has score work to cover the exp latency
                pending = []
                for c in range(nkv):
                    # diagonal chunks only touch q columns >= qlo; computing
                    # (and exp-ing) the dead region would be wasted work
                    qlo = max(0, c * P - qt * QT)
                    W = QT - qlo
                    sp2 = ps.tile([P, 2, QT], F32, tag="scores", bufs=2,
                                  name=f"sp{b}_{qt}_{c}")
                    for h in range(2):
                        nc.tensor.matmul(
                            sp2[:, h, qlo:],
                            kT_sb[b][h * HD:(h + 1) * HD, c * P:(c + 1) * P],
                            qT_sb[b][h * HD:(h + 1) * HD,
                                     qt * QT + qlo:(qt + 1) * QT])
                    ex2 = exps.tile([P, 2, QT], BF16, tag="ex",
                                    name=f"ex{b}_{qt}_{c}")
                    nc.scalar.activation(ex2[:, :, qlo:], sp2[:, :, qlo:],
                                         AF.Exp, scale=0.125)
                    if qlo or c == 4 * qt:
                        mk = mask_sb[:, 512:1024 - qlo]
                        nc.vector.tensor_mul(
                            ex2[:, :, qlo:], ex2[:, :, qlo:],
                            mk.rearrange("p (o w) -> p o w", o=1)
                              .to_broadcast((P, 2, W)))
                    pending.append((c, ex2))
                    if len(pending) > 4:
                        emit_out(*pending.pop(0))
                for item in pending:
                    emit_out(*item)
                for h in range(2):
                    rec = work.tile([1, QT], F32, tag="rec")
                    nc.vector.reciprocal(rec[:], op[h][HD:HD + 1, :])
                    gt = work.tile([1, QT], F32, tag="gatet", bufs=3,
                                   name=f"gt{b}_{qt}_{h}")
                    nc.sync.dma_start(
                        gt[:], gateT[h:h + 1,
                                     b * S + qt * QT:b * S + (qt + 1) * QT])
                    f = work.tile([1, QT], F32, tag="f")
                    nc.vector.tensor_mul(f[:], rec[:], gt[:])
                    fp = ps.tile([HD, QT], F32, tag="wop", bufs=2,
                                 name=f"fp{b}_{qt}_{h}")
                    nc.tensor.matmul(fp[:], ones_sb[:], f[:])
                    fs = work.tile([HD, QT], BF16, tag="fs")
                    nc.vector.tensor_copy(fs[:], fp[:])
                    nc.vector.tensor_mul(attnT2[h * HD:(h + 1) * HD, :],
                                         op[h][0:HD, :], fs[:])
                for sub in range(QT // P):
                    r0 = b * S + qt * QT + sub * P
                    pos = work.tile([P, D], F32, tag="pos", bufs=4,
                                    name=f"pos{b}_{qt}_{sub}")
                    for n in range(2):
                        wop = ps.tile([P, 512], F32, tag="wop", bufs=2,
                                      name=f"wop{b}_{qt}_{sub}_{n}")
                        nc.tensor.matmul(
                            wop[:], attnT2[:, sub * P:(sub + 1) * P],
                            wo_sb[:, n * 512:(n + 1) * 512])
                        # 3:1 DVE:ACT eviction split keeps the engines level
                        if (sub * 2 + n) % 4 == 3:
                            nc.scalar.copy(pos[:, n * 512:(n + 1) * 512],
                                           wop[:])
                        else:
                            nc.vector.tensor_copy(pos[:, n * 512:(n + 1) * 512],
                                                  wop[:])
                    nc.sync.dma_start(po[r0:r0 + P, :], pos[:])

    nc.compile()
    return nc


# --------------------------------------------------------------------------
# Launch B builder: expert-parallel MoE + token-sharded shared MLP
# --------------------------------------------------------------------------
def _mm_slices(n, step=512):
    out, o = [], 0
    while o < n:
        out.append(slice(o, min(o + step, n)))
        o += step
    return out


def _build_moe(C):
    assert C % P == 0
    nc = bacc.Bacc(None, target_bir_lowering=False, debug=False)

    xeT = nc.declare_dram_parameter("xeT", [D, C], BF16, isOutput=False)
    wug = nc.declare_dram_parameter("wug", [D, 2 * I], BF16, isOutput=False)
    wdn = nc.declare_dram_parameter("wdn", [I, D], BF16, isOutput=False)
    cvec = nc.declare_dram_parameter("cvec", [C, 1], F32, isOutput=False)
    hnT = nc.declare_dram_parameter("hnT", [D, T // 8], BF16, isOutput=False)
    wsh = nc.declare_dram_parameter("wsh", [D, 2 * ISH], BF16, isOutput=False)
    wdsh = nc.declare_dram_parameter("wdsh", [ISH, D], BF16, isOutput=False)
    ye = nc.declare_dram_parameter("ye", [C, D], F32, isOutput=True)
    ysh = nc.declare_dram_parameter("ysh", [T // 8, D], F32, isOutput=True)

    with tile.TileContext(nc) as tc, ExitStack() as ctx:
        const = ctx.enter_context(tc.tile_pool(name="const", bufs=1))
        psum_g = ctx.enter_context(tc.tile_pool(name="psum_g", bufs=1, space="PSUM"))
        psum_d = ctx.enter_context(tc.tile_pool(name="psum_d", bufs=1, space="PSUM"))
        acts = ctx.enter_context(tc.tile_pool(name="acts", bufs=1))
        stage = ctx.enter_context(tc.tile_pool(name="stage", bufs=3))

        def load_split(name, dram, cols):
            t = const.tile([P, 8, cols], BF16, name=name)
            r = dram.rearrange("(a p) c -> p a c", p=P)
            for d in range(8):
                nc.sync.dma_start(t[:, d], r[:, d])
            return t

        # the d-loop of the first up/gate matmul group consumes (xeT_d, wug_d)
        # pairs in order, so issue those DMA descriptors first, interleaved
        xeT_sb = const.tile([P, 8, C], BF16, name="xeT_sb")
        xeT_r = xeT.rearrange("(a p) c -> p a c", p=P)
        wug_sb = const.tile([P, 8, 2 * I], BF16, name="wug_sb")
        wug_r = wug.rearrange("(a p) c -> p a c", p=P)
        for d in range(8):
            nc.sync.dma_start(xeT_sb[:, d], xeT_r[:, d])
            nc.gpsimd.dma_start(wug_sb[:, d], wug_r[:, d])
        wdn_sb = load_split("wdn_sb", wdn, D)
        cv_sb = const.tile([P, C // P], F32)
        nc.sync.dma_start(cv_sb[:], cvec.rearrange("(a p) one -> p (a one)", p=P))
        hnT_sb = load_split("hnT_sb", hnT, T // 8)
        wsh_sb = load_split("wsh_sb", wsh, 2 * ISH)
        wdsh_sb = load_split("wdsh_sb", wdsh, D)

        def glu_phase(xT_sb_, w_sb_, n_free, name):
            # token-slice OUTER loop: the first token block of every i-chunk
            # finishes early, so the down-projection overlaps the rest of
            # the GLU instead of waiting for all of it
            slices = _mm_slices(n_free)
            act_tiles = [acts.tile([P, n_free], BF16, tag=f"act{n}",
                                   name=f"{name}_a{n}") for n in range(8)]
            for sl in slices:
                for n in range(8):
                    pg = psum_g.tile([P, sl.stop - sl.start], F32, tag="pg",
                                     bufs=3, name=f"{name}_pg{n}_{sl.start}")
                    pu = psum_g.tile([P, sl.stop - sl.start], F32, tag="pu",
                                     bufs=3, name=f"{name}_pu{n}_{sl.start}")
                    for d in range(8):
                        lg = w_sb_[:, d, n * P:(n + 1) * P]
                        lu = w_sb_[:, d, I + n * P:I + (n + 1) * P]
                        nc.tensor.matmul(pg[:], lg, xT_sb_[:, d, sl],
                                         start=(d == 0), stop=(d == 7))
                        nc.tensor.matmul(pu[:], lu, xT_sb_[:, d, sl],
                                         start=(d == 0), stop=(d == 7))
                    sg = acts.tile([P, sl.stop - sl.start], BF16, tag="sg",
                                   bufs=3, name=f"{name}_sg{n}_{sl.start}")
                    nc.scalar.activation(sg[:], pg[:], AF.Silu)
                    nc.vector.tensor_mul(act_tiles[n][:, sl], pu[:], sg[:])
            return act_tiles

        def down_phase(act_tiles, wd_sb_, out_dram, n_tok, name, scale_sb=None):
            for t in range(n_tok // P):
                ysb = stage.tile([P, D], F32, tag="ysb", bufs=4, name=f"{name}_ysb{t}")
                for half in range(2):
                    py = psum_d.tile([P, 512], F32, tag="py", bufs=2,
                                     name=f"{name}_py{t}_{half}")
                    for i in range(8):
                        lhsT = act_tiles[i][:, t * P:(t + 1) * P]
                        nc.tensor.matmul(py[:], lhsT,
                                         wd_sb_[:, i, half * 512:(half + 1) * 512],
                                         start=(i == 0), stop=(i == 7))
                    dst = ysb[:, half * 512:(half + 1) * 512]
                    if scale_sb is not None:
                        nc.scalar.activation(dst, py[:], AF.Copy,
                                             scale=scale_sb[:, t:t + 1])
                    else:
                        nc.scalar.copy(dst, py[:])
                nc.sync.dma_start(out_dram[t * P:(t + 1) * P, :], ysb[:])

        act_e = glu_phase(xeT_sb, wug_sb, C, "e")
        act_s = glu_phase(hnT_sb, wsh_sb, T // 8, "s")
        down_phase(act_e, wdn_sb, ye, C, "e", scale_sb=cv_sb)
        down_phase(act_s, wdsh_sb, ysh, T // 8, "s")

    nc.compile()
    return nc


# --------------------------------------------------------------------------
# Host orchestration
# --------------------------------------------------------------------------
def _rms_norm(x, w):
    var = np.mean(np.square(x), axis=-1, keepdims=True)
    return (x / np.sqrt(var + EPS)) * w


def kernel(x, ln1_w, ln2_w, w_q, w_k, w_v, w_o, attn_gate,
           router, w_up_gate, w_down_moe, w_gate_sh, w_up_sh, w_down_sh):
    x = np.asarray(x, np.float32)
    core_ids = list(range(N_CORES))

    # ---- host prep for launch A
    x_flat = x.reshape(T, D)
    xn = _rms_norm(x_flat, np.asarray(ln1_w, np.float32))
    xT = _bf16(np.ascontiguousarray(xn.T))

    half = ROT_DIM // 2
    inv_freq = 1.0 / ROPE_THETA ** (np.arange(half, dtype=np.float32) / half)
    ang = np.arange(S, dtype=np.float32)[:, None] * inv_freq[None, :]
    cos3 = np.tile(np.cos(ang), (1, 3)).astype(np.float32)
    sin3 = np.tile(np.sin(ang), (1, 3)).astype(np.float32)
    mask = _bf16((np.arange(1024)[None, :] - 512 >= np.arange(P)[:, None])
                 .astype(np.float32))
    gate_full = 2.0 / (1.0 + np.exp(-(xn[:, :G] @ np.asarray(attn_gate, np.float32))))

    w_q = np.asarray(w_q, np.float32)
    w_k = np.asarray(w_k, np.float32)
    w_v = np.asarray(w_v, np.float32)
    w_o = np.asarray(w_o, np.float32)

    if "attn" not in _cache:
        _cache["attn"] = _build_attn()
    ncA = _cache["attn"]

    in_maps = []
    for c in core_ids:
        h0, kv = 2 * c, c // 2
        wpack = np.concatenate(
            [w_q[:, h0 * HD:(h0 + 2) * HD],
             w_k[:, kv * HD:(kv + 1) * HD],
             w_v[:, kv * HD:(kv + 1) * HD]], axis=1)
        gateT = np.ascontiguousarray(gate_full[:, h0:h0 + 2].T).astype(np.float32)
        in_maps.append(dict(
            xT=xT, wpack=_bf16(wpack), wo=_bf16(w_o[h0 * HD:(h0 + 2) * HD, :]),
            gateT=gateT, cos3=cos3, sin3=sin3, mask=mask))

    resA = run_bass_kernel_spmd(ncA, in_maps, core_ids)

    attn_out = np.zeros((T, D), np.float32)
    for c in core_ids:
        attn_out += resA.results[c]["po"]

    # ---- host routing + dispatch
    h = x_flat + attn_out
    hn = _rms_norm(h, np.asarray(ln2_w, np.float32))
    logits = (hn @ np.asarray(router, np.float32)).astype(np.float32)
    logits -= logits.max(-1, keepdims=True)
    pe = np.exp(logits)
    probs = pe / pe.sum(-1, keepdims=True)
    order = np.argsort(-probs, axis=-1, kind="stable")
    sel = order[:, :K]                          # [T, K]
    wsel = np.take_along_axis(probs, sel, -1)   # [T, K]
    wsel = wsel / wsel.sum(-1, keepdims=True)

    idx_e, cw_e = [], []
    for e in range(E):
        hit = (sel == e)
        tok = np.nonzero(hit.any(-1))[0]
        w = (wsel * hit).sum(-1)[tok]
        idx_e.append(tok)
        cw_e.append(w.astype(np.float32))
    maxc = max(len(t) for t in idx_e)
    C = max(P, ((maxc + P - 1) // P) * P)

    if ("moe", C) not in _cache:
        _cache[("moe", C)] = _build_moe(C)
    ncB = _cache[("moe", C)]

    hnT_b = _bf16(np.ascontiguousarray(hn.T))
    w_up_gate = np.asarray(w_up_gate, np.float32)
    w_down_moe = np.asarray(w_down_moe, np.float32)
    wsh_full = _bf16(np.concatenate(
        [np.asarray(w_gate_sh, np.float32), np.asarray(w_up_sh, np.float32)], axis=1))
    wdsh_full = _bf16(np.asarray(w_down_sh, np.float32))

    in_maps_b = []
    for e in range(E):
        tok = idx_e[e]
        xe = np.zeros((D, C), ml_dtypes.bfloat16)
        xe[:, :len(tok)] = hnT_b[:, tok]
        cv = np.zeros((C, 1), np.float32)
        cv[:len(tok), 0] = cw_e[e]
        in_maps_b.append(dict(
            xeT=xe, wug=_bf16(w_up_gate[e]), wdn=_bf16(w_down_moe[e]), cvec=cv,
            hnT=np.ascontiguousarray(hnT_b[:, e * (T // 8):(e + 1) * (T // 8)]),
            wsh=wsh_full, wdsh=wdsh_full))

    resB = run_bass_kernel_spmd(ncB, in_maps_b, core_ids)

    out = h.copy()
    for e in range(E):
        tok = idx_e[e]
        out[tok] += resB.results[e]["ye"][:len(tok)]
        out[e * (T // 8):(e + 1) * (T // 8)] += resB.results[e]["ysh"]

    return out.reshape(B, S, D).astype(np.float32)



# revision 9
# speedup vs baseline: 1.4160x; 1.4160x over previous
"""Trainium2 8-core kernel for nn_Block_47794396070541 (attention + top-2 MoE +
shared MLP transformer block).

Three SPMD launches (full inputs in, full output out; host does the cheap glue):

Launch A (attention, tensor-parallel over heads): each core owns 2 of 16
  q-heads (plus their shared kv head) for both batches. fp8 DoubleRow QKV
  projection, bf16 causal scores, exp -> fp8 probs (bias -4 keeps them in
  fp8e4 range), fp8 DoubleRow transposed-PV with an appended ones column for
  the softmax denominator, reciprocal*gate folded into the eviction scale.
  Output: per-head gated attention [T, 2, 64] fp8 (x16), pre-w_o.

Launch B1 (w_o, token-parallel): core c computes attn[T/8 shard] @ w_o with
  fp8 DoubleRow from the host-assembled [NH*HD, T] fp8 transpose. The host
  adds x and rms-norms to get h / hn exactly in f32.

Host: router softmax + top-2 + dispatch (tiny), all in f32.

Launch B2 (MoE expert-parallel + token-sharded shared MLP): all GEMMs fp8
  DoubleRow with weights pre-scaled x64 (host, power of two); act8 stores
  8*silu(g)*u in fp8; combine weights + descale folded into the host
  scatter-add.

fp8 K-tile SBUF layout everywhere: [128, KT, 2, cols], K = (kt*2+two)*128+p,
so one DoubleRow matmul contracts 256 at 0.5 cycles/row.
"""

from contextlib import ExitStack

import numpy as np
import ml_dtypes

import concourse.mybir as mybir
import concourse.tile as tile
from concourse import bacc
from concourse.bass_utils import run_bass_kernel_spmd
from concourse.masks import make_identity

F32 = mybir.dt.float32
BF16 = mybir.dt.bfloat16
FP8 = mybir.dt.float8e4
AF = mybir.ActivationFunctionType
ALU = mybir.AluOpType
DR = mybir.MatmulPerfMode.DoubleRow

# problem shapes
B, S, D = 2, 2048, 1024
T = B * S
NH, MH, HD = 16, 4, 64
G = 12
E, K, I = 8, 2, 1024
ISH = 1024
EPS = 1e-5
QK_EPS = 1e-6
ROPE_THETA = 1024.0
ROT_DIM = 32
P = 128
N_CORES = 8
TSH = T // N_CORES          # 512 tokens per core for shared MLP / w_o
WSC = 64.0                  # fp8 weight scale (power of 2)
ASC = 8.0                   # fp8 activation scale for MoE act8
POSC = 16.0                 # fp8 scale of launch-A attention output

_cache = {}


def _fp8(a):
    return np.ascontiguousarray(a).astype(ml_dtypes.float8_e4m3)


def _bf16(a):
    return np.ascontiguousarray(a).astype(ml_dtypes.bfloat16)


# --------------------------------------------------------------------------
# Launch A: attention (2 q-heads per core, both batches), pre-w_o output
# --------------------------------------------------------------------------
def _build_attn():
    nc = bacc.Bacc(None, target_bir_lowering=False, debug=False)

    xT8 = nc.declare_dram_parameter("xT8", [D, T], FP8, isOutput=False)
    wp8 = nc.declare_dram_parameter("wp8", [D, 256], FP8, isOutput=False)
    cos3 = nc.declare_dram_parameter("cos3", [S, 48], BF16, isOutput=False)
    sin3 = nc.declare_dram_parameter("sin3", [S, 48], BF16, isOutput=False)
    gate = nc.declare_dram_parameter("gate", [T, 2], F32, isOutput=False)
    maskadd = nc.declare_dram_parameter("maskadd", [P, P], BF16, isOutput=False)
    po = nc.declare_dram_parameter("po", [T, 2, HD], FP8, isOutput=True)

    SC = S // P               # 16 kv chunks per batch
    NPAIR = SC // 2

    with tile.TileContext(nc) as tc, ExitStack() as ctx:
        const = ctx.enter_context(tc.tile_pool(name="const", bufs=1))
        work = ctx.enter_context(tc.tile_pool(name="work", bufs=2))
        exps = ctx.enter_context(tc.tile_pool(name="exps", bufs=14))
        outp = ctx.enter_context(tc.tile_pool(name="outp", bufs=3))

        # ---- input DMAs (wp8 + first xT8 k-tile first: the projection's
        # d-accumulation consumes kt in order)
        wp_sb = const.tile([P, 4, 2, 256], FP8)
        nc.sync.dma_start(wp_sb[:], wp8.rearrange("(kt two p) c -> p kt two c",
                                                  p=P, two=2))
        xT_sb = const.tile([P, 4, 2, T], FP8)
        xT_r = xT8.rearrange("(kt two p) c -> p kt two c", p=P, two=2)
        for kt in range(4):
            eng = nc.sync if kt % 2 == 0 else nc.gpsimd
            eng.dma_start(xT_sb[:, kt, :, 0:T // 2], xT_r[:, kt, :, 0:T // 2])
        for kt in range(4):
            eng = nc.sync if kt % 2 == 0 else nc.gpsimd
            eng.dma_start(xT_sb[:, kt, :, T // 2:], xT_r[:, kt, :, T // 2:])
        cos_sb = const.tile([P, SC, 3, 16], BF16)
        nc.sync.dma_start(cos_sb[:], cos3.rearrange("(a p) (g j) -> p a g j",
                                                    p=P, g=3))
        sin_sb = const.tile([P, SC, 3, 16], BF16)
        nc.sync.dma_start(sin_sb[:], sin3.rearrange("(a p) (g j) -> p a g j",
                                                    p=P, g=3))
        gate_sb = const.tile([P, T // P, 2], F32)
        nc.sync.dma_start(gate_sb[:], gate.rearrange("(a p) h -> p a h", p=P))
        mask_sb = const.tile([P, P], BF16)
        nc.sync.dma_start(mask_sb[:], maskadd[:])
        identb = const.tile([P, P], BF16)
        make_identity(nc, identb[:])
        eps_sb = const.tile([P, 1], F32)
        nc.vector.memset(eps_sb[:], 64.0 * 64.0 * QK_EPS)
        deps_sb = const.tile([P, 1], F32)
        nc.vector.memset(deps_sb[:], 1e-4)
        ones_sb = const.tile([P, HD], BF16)
        nc.vector.memset(ones_sb[:], 1.0)
        deps2_sb = const.tile([P, 2], F32)
        nc.vector.memset(deps2_sb[:], 1e-4)
        nbias_sb = const.tile([P, 1], F32)
        nc.vector.memset(nbias_sb[:], -4.0)

        qT_sb = [const.tile([P, S], BF16, name=f"qT{b}") for b in range(B)]
        kT_sb = [const.tile([P, S], BF16, name=f"kT{b}") for b in range(B)]
        # v8: [128, chunk(16), 65]; col 64 = ones (softmax denominator)
        v8_sb = [const.tile([P, SC, HD + 1], FP8, name=f"v8{b}") for b in range(B)]
        for b in range(B):
            nc.gpsimd.memset(v8_sb[b][:, :, HD:HD + 1], 1.0)

        # ---- phase 1: projection + qk-norm + rope + transposes
        ph1_cm = tc.tile_pool(name="ph1", bufs=1, space="PSUM")
        ph1 = ph1_cm.__enter__()
        for it in range(8):               # 4 chunks of 128 tokens each
            b, c0 = it // 4, (it % 4) * 4
            t0 = it * 512
            pp = ph1.tile([P, 4, 4, HD], F32, tag="pp", bufs=2, name=f"pp{it}")
            for j in range(4):
                for kt in range(4):
                    nc.tensor.matmul(pp[:, j],
                                     xT_sb[:, kt, :, t0 + j * P:t0 + (j + 1) * P],
                                     wp_sb[:, kt],
                                     start=(kt == 0), stop=(kt == 3),
                                     perf_mode=DR)
            pp_sb = work.tile([P, 4, 4, HD], BF16, tag="pp_sb", name=f"ppsb{it}")
            nc.vector.tensor_copy(pp_sb[:], pp[:])
            sq = work.tile([P, 4, 3, HD], BF16, tag="sq", name=f"sq{it}")
            nc.scalar.activation(sq[:], pp[:, :, 0:3], AF.Square)
            ssum = work.tile([P, 4, 3], F32, tag="ssum", name=f"ssum{it}")
            nc.vector.reduce_sum(ssum[:], sq[:], axis=mybir.AxisListType.X)
            rstd = work.tile([P, 4, 3, 1], BF16, tag="rstd", name=f"rstd{it}")
            nc.scalar.activation(rstd[:], ssum[:], AF.Sqrt, scale=1.0 / 64,
                                 bias=eps_sb[:])
            with nc.allow_low_precision(reason="bf16 norm scale is plenty"):
                nc.vector.reciprocal(rstd[:], rstd[:])
            qkv = work.tile([P, 4, 3, HD], BF16, tag="qkv", name=f"qkv{it}")
            nc.vector.tensor_mul(qkv[:], pp_sb[:, :, 0:3],
                                 rstd[:].to_broadcast((P, 4, 3, HD)))
            # rope (linear, commutes with the rstd scaling)
            x1 = qkv[:, :, :, 0:16]
            x2 = qkv[:, :, :, 16:32]
            cs = cos_sb[:, c0:c0 + 4]
            sn = sin_sb[:, c0:c0 + 4]
            tmp = work.tile([P, 4, 4, 3, 16], BF16, tag="ropetmp", name=f"rt{it}")
            nc.gpsimd.tensor_mul(tmp[:, 0], x1, cs)
            nc.gpsimd.tensor_mul(tmp[:, 1], x2, sn)
            nc.gpsimd.tensor_mul(tmp[:, 2], x2, cs)
            nc.gpsimd.tensor_mul(tmp[:, 3], x1, sn)
            nc.vector.tensor_sub(x1, tmp[:, 0], tmp[:, 1])
            nc.vector.tensor_add(x2, tmp[:, 2], tmp[:, 3])
            # v8 = pp_v / 64 (fp8)
            nc.vector.scalar_tensor_tensor(
                v8_sb[b][:, c0:c0 + 4, 0:HD], pp[:, :, 3], 1.0 / 64,
                ones_sb[:].rearrange("p (o w) -> p o w", o=1)
                          .to_broadcast((P, 4, HD)), ALU.mult, ALU.mult)
            # transposes -> qT [2 heads x 64, q], kT [64, q]
            tq = ph1.tile([P, 4, P], BF16, tag="tq", bufs=2, name=f"tq{it}")
            tk = ph1.tile([HD, 4, P], BF16, tag="tk", bufs=2, name=f"tk{it}")
            for j in range(4):
                nc.tensor.transpose(tq[:, j], qkv[:, j, 0:2], identb[:])
                nc.tensor.transpose(tk[:, j], qkv[:, j, 2], identb[:])
            sl = slice(c0 * P, (c0 + 4) * P)
            nc.vector.tensor_copy(qT_sb[b][:, sl],
                                  tq[:].rearrange("p a q -> p (a q)"))
            nc.vector.tensor_copy(kT_sb[b][0:HD, sl],
                                  tk[:].rearrange("p a q -> p (a q)"))
            if it % 4 == 3:   # batch done: duplicate kv head into rows 64-127
                nc.gpsimd.dma_start(kT_sb[b][HD:P, :], kT_sb[b][0:HD, :])
        ph1_cm.__exit__(None, None, None)

        # ---- phase 2: scores -> exp(fp8) -> DoubleRow PV -> scaled evict
        ps = ctx.enter_context(tc.tile_pool(name="ps", bufs=1, space="PSUM"))
        QT = 512

        def emit_scores(b, qt):
            nkv = 4 * qt + 4
            ex_tiles = []
            for c in range(nkv):
                qlo = max(0, c * P - qt * QT)
                if c % 2 == 0:
                    ex = exps.tile([P, 2, 2, QT], FP8, tag="ex",
                                   name=f"ex{b}_{qt}_{c}")
                    ex_tiles.append(ex)
                sp = ps.tile([P, 2, QT], F32, tag="sp", bufs=3,
                             name=f"sp{b}_{qt}_{c}")
                for h in range(2):
                    nc.tensor.matmul(
                        sp[:, h, qlo:],
                        kT_sb[b][h * HD:(h + 1) * HD, c * P:(c + 1) * P],
                        qT_sb[b][h * HD:(h + 1) * HD,
                                 qt * QT + qlo:(qt + 1) * QT])
                if c >= 4 * qt:   # chunk containing the causal diagonal
                    nc.vector.tensor_add(
                        sp[:, :, qlo:qlo + P], sp[:, :, qlo:qlo + P],
                        mask_sb[:].rearrange("p (o w) -> p o w", o=1)
                               .to_broadcast((P, 2, P)))
                if qlo:   # zero the never-exp'd left region of this slot
                    nc.vector.memset(ex[:, c % 2, :, 0:qlo], 0.0)
                nc.scalar.activation(ex[:, c % 2, :, qlo:], sp[:, :, qlo:],
                                     AF.Exp, scale=0.125, bias=nbias_sb[:])
            return ex_tiles

        def emit_pv(b, qt, ex_tiles):
            for i2 in range(2):           # two double-sub-chunk groups
                op = ps.tile([P, 2, 2, HD + 2], F32, tag="op", bufs=2,
                             name=f"op{b}_{qt}_{i2}")
                for s in range(2):
                    ig = 4 * qt + i2 * 2 + s
                    npr = ig // 2 + 1
                    qsl = slice((i2 * 2 + s) * P, (i2 * 2 + s + 1) * P)
                    for h in range(2):
                        for pr in range(npr):
                            nc.tensor.matmul(
                                op[:, s, h, 0:HD + 1],
                                ex_tiles[pr][:, :, h, qsl],
                                v8_sb[b][:, 2 * pr:2 * pr + 2, :],
                                start=(pr == 0), stop=(pr == npr - 1),
                                perf_mode=DR)
                gch = b * 16 + qt * 4 + i2 * 2
                den = work.tile([P, 2, 2], F32, tag="den", name=f"den{b}{qt}{i2}")
                nc.vector.scalar_tensor_tensor(
                    den[:], op[:, :, :, HD], 1.0,
                    deps2_sb[:].rearrange("p (o w) -> p o w", o=1)
                               .to_broadcast((P, 2, 2)), ALU.mult, ALU.add)
                rec = work.tile([P, 2, 2], F32, tag="rec", name=f"rec{b}{qt}{i2}")
                nc.vector.reciprocal(rec[:], den[:])
                fg = work.tile([P, 2, 2, 1], F32, tag="fg", name=f"fg{b}{qt}{i2}")
                nc.vector.scalar_tensor_tensor(
                    fg[:, :, :, 0], rec[:], POSC,
                    gate_sb[:, gch:gch + 2], ALU.mult, ALU.mult)
                po_sb = outp.tile([P, 2, 2, HD], FP8, tag="po_sb",
                                  name=f"po{b}{qt}{i2}")
                nc.vector.tensor_mul(po_sb[:], op[:, :, :, 0:HD],
                                     fg[:].to_broadcast((P, 2, 2, HD)))
                nc.gpsimd.dma_start(
                    po.rearrange("(a p) h d -> p a h d", p=P)[:, gch:gch + 2],
                    po_sb[:])

        combos = [(b, qt) for b in range(B) for qt in range(S // QT)]
        pending = None
        for (b, qt) in combos:
            ex_tiles = emit_scores(b, qt)
            if pending is not None:
                emit_pv(*pending)
            pending = (b, qt, ex_tiles)
        emit_pv(*pending)

    nc.compile()
    return nc


# --------------------------------------------------------------------------
# Launch B1: w_o product for a T/8 token shard (fp8 DoubleRow)
# --------------------------------------------------------------------------
def _build_wo():
    nc = bacc.Bacc(None, target_bir_lowering=False, debug=False)

    aT8 = nc.declare_dram_parameter("aT8", [NH * HD, TSH], FP8, isOutput=False)
    wo8 = nc.declare_dram_parameter("wo8", [NH * HD, D], FP8, isOutput=False)
    wout = nc.declare_dram_parameter("wout", [TSH, D], BF16, isOutput=True)

    with tile.TileContext(nc) as tc, ExitStack() as ctx:
        const = ctx.enter_context(tc.tile_pool(name="const", bufs=1))
        ps = ctx.enter_context(tc.tile_pool(name="ps", bufs=1, space="PSUM"))
        stage = ctx.enter_context(tc.tile_pool(name="stage", bufs=3))

        a_sb = const.tile([P, 4, 2, TSH], FP8)
        a_r = aT8.rearrange("(kt two p) c -> p kt two c", p=P, two=2)
        w_sb = const.tile([P, 4, 2, D], FP8)
        w_r = wo8.rearrange("(kt two p) c -> p kt two c", p=P, two=2)
        for kt in range(4):
            nc.sync.dma_start(a_sb[:, kt], a_r[:, kt])
            nc.gpsimd.dma_start(w_sb[:, kt], w_r[:, kt])

        for t in range(TSH // P):
            ysb = stage.tile([P, D], BF16, tag="ysb", name=f"ysb{t}")
            for dh in range(2):
                py = ps.tile([P, 512], F32, tag="py", bufs=3, name=f"py{t}{dh}")
                for q in range(2):
                    dsl = slice(dh * 512 + q * 256, dh * 512 + (q + 1) * 256)
                    for kt in range(4):
                        nc.tensor.matmul(py[:, q * 256:(q + 1) * 256],
                                         a_sb[:, kt, :, t * P:(t + 1) * P],
                                         w_sb[:, kt, :, dsl],
                                         start=(kt == 0), stop=(kt == 3),
                                         perf_mode=DR)
                if dh == 0:
                    nc.scalar.copy(ysb[:, 0:512], py[:])
                else:
                    nc.vector.tensor_copy(ysb[:, 512:D], py[:])
            nc.sync.dma_start(wout[t * P:(t + 1) * P, :], ysb[:])

    nc.compile()
    return nc


# --------------------------------------------------------------------------
# Launch B2: expert-parallel MoE + token-sharded shared MLP (fp8 DoubleRow)
# --------------------------------------------------------------------------
def _build_moe(C):
    assert C % 256 == 0
    nc = bacc.Bacc(None, target_bir_lowering=False, debug=False)

    xeT = nc.declare_dram_parameter("xeT", [D, C], FP8, isOutput=False)
    wug = nc.declare_dram_parameter("wug", [D, 2 * I], FP8, isOutput=False)
    wdn = nc.declare_dram_parameter("wdn", [I, D], FP8, isOutput=False)
    hnT = nc.declare_dram_parameter("hnT", [D, TSH], FP8, isOutput=False)
    wsh = nc.declare_dram_parameter("wsh", [D, 2 * ISH], FP8, isOutput=False)
    wdsh = nc.declare_dram_parameter("wdsh", [ISH, D], FP8, isOutput=False)
    ye = nc.declare_dram_parameter("ye", [C, D], BF16, isOutput=True)
    ysh = nc.declare_dram_parameter("ysh", [TSH, D], BF16, isOutput=True)

    with tile.TileContext(nc) as tc, ExitStack() as ctx:
        const = ctx.enter_context(tc.tile_pool(name="const", bufs=1))
        psg = ctx.enter_context(tc.tile_pool(name="psg", bufs=1, space="PSUM"))
        psd = ctx.enter_context(tc.tile_pool(name="psd", bufs=1, space="PSUM"))
        acts = ctx.enter_context(tc.tile_pool(name="acts", bufs=1))
        sgp = ctx.enter_context(tc.tile_pool(name="sgp", bufs=3))

        def kt_layout(name, dram, cols, queue):
            kt = dram.shape[0] // 256
            t = const.tile([128, kt, 2, cols], FP8, name=name)
            r = dram.rearrange("(kt two p) c -> p kt two c", p=128, two=2)
            for k in range(kt):
                queue.dma_start(t[:, k], r[:, k])
            return t

        # first GLU psum needs all-kt of xeT token-tile 0 + wug col-block 0
        xeT_sb = const.tile([128, 4, 2, C], FP8, name="xeT_sb")
        xeT_r = xeT.rearrange("(kt two p) c -> p kt two c", p=128, two=2)
        wug_sb = const.tile([128, 4, 2, 2 * I], FP8, name="wug_sb")
        wug_r = wug.rearrange("(kt two p) c -> p kt two c", p=128, two=2)
        T0 = min(512, C)
        for k in range(4):
            nc.sync.dma_start(xeT_sb[:, k, :, 0:T0], xeT_r[:, k, :, 0:T0])
            nc.gpsimd.dma_start(wug_sb[:, k, :, 0:128], wug_r[:, k, :, 0:128])
            nc.gpsimd.dma_start(wug_sb[:, k, :, I:I + 128], wug_r[:, k, :, I:I + 128])
        for k in range(4):
            nc.gpsimd.dma_start(wug_sb[:, k, :, 128:I], wug_r[:, k, :, 128:I])
            nc.gpsimd.dma_start(wug_sb[:, k, :, I + 128:], wug_r[:, k, :, I + 128:])
            if T0 < C:
                nc.sync.dma_start(xeT_sb[:, k, :, T0:], xeT_r[:, k, :, T0:])
        wdn_sb = kt_layout("wdn_sb", wdn, D, nc.sync)
        hnT_sb = kt_layout("hnT_sb", hnT, TSH, nc.sync)
        wsh_sb = kt_layout("wsh_sb", wsh, 2 * ISH, nc.sync)
        wdsh_sb = kt_layout("wdsh_sb", wdsh, D, nc.sync)

        def glu_down(xT_sb_, w_sb_, wd_sb_, out_dram, n_tok, name):
            # token-tile outer (512), I-col blocks (128) inner; down-proj per
            # token tile follows its full column sweep. act8 [128, kt, two, tok]
            nt = (n_tok + 511) // 512
            act8 = acts.tile([128, 4, 2, n_tok], FP8, name=f"{name}_act8")
            for t in range(nt):
                ts = slice(t * 512, min((t + 1) * 512, n_tok))
                W = ts.stop - ts.start
                for n in range(8):
                    pg = psg.tile([128, 512], F32, tag="pg", bufs=2,
                                  name=f"{name}_pg{t}_{n}")
                    pu = psg.tile([128, 512], F32, tag="pu", bufs=2,
                                  name=f"{name}_pu{t}_{n}")
                    for half in range(W // 256):
                        hs = slice(ts.start + half * 256, ts.start + (half + 1) * 256)
                        po = slice(half * 256, (half + 1) * 256)
                        for kt in range(4):
                            nc.tensor.matmul(
                                pg[:, po], w_sb_[:, kt, :, n * 128:(n + 1) * 128],
                                xT_sb_[:, kt, :, hs],
                                start=(kt == 0), stop=(kt == 3), perf_mode=DR)
                        for kt in range(4):
                            nc.tensor.matmul(
                                pu[:, po], w_sb_[:, kt, :, I + n * 128:I + (n + 1) * 128],
                                xT_sb_[:, kt, :, hs],
                                start=(kt == 0), stop=(kt == 3), perf_mode=DR)
                    sg = sgp.tile([128, 512], BF16, tag="sg", name=f"{name}_sg{t}_{n}")
                    nc.scalar.activation(sg[:, 0:W], pg[:, 0:W], AF.Silu,
                                         scale=1.0 / WSC)
                    nc.vector.scalar_tensor_tensor(
                        act8[:, n // 2, n % 2, ts], pu[:, 0:W], ASC / WSC,
                        sg[:, 0:W], ALU.mult, ALU.mult)
                for s in range(W // 128):
                    t0 = ts.start + s * 128
                    ysb = sgp.tile([128, 1024], BF16, tag="ysb", bufs=3,
                                   name=f"{name}_ysb{t}_{s}")
                    for dh in range(2):
                        py = psd.tile([128, 512], F32, tag="py", bufs=3,
                                      name=f"{name}_py{t}_{s}_{dh}")
                        for q in range(2):
                            dsl = slice(dh * 512 + q * 256, dh * 512 + (q + 1) * 256)
                            for kt in range(4):
                                nc.tensor.matmul(
                                    py[:, q * 256:(q + 1) * 256],
                                    act8[:, kt, :, t0:t0 + 128],
                                    wd_sb_[:, kt, :, dsl],
                                    start=(kt == 0), stop=(kt == 3), perf_mode=DR)
                        dst = ysb[:, dh * 512:(dh + 1) * 512]
                        if dh == 0:
                            nc.scalar.copy(dst, py[:])
                        else:
                            nc.vector.tensor_copy(dst, py[:])
                    nc.gpsimd.dma_start(out_dram[t0:t0 + 128, :], ysb[:])

        glu_down(xeT_sb, wug_sb, wdn_sb, ye, C, "e")
        glu_down(hnT_sb, wsh_sb, wdsh_sb, ysh, TSH, "s")

    nc.compile()
    return nc


# --------------------------------------------------------------------------
# Host orchestration
# --------------------------------------------------------------------------
def _rms_norm(x, w):
    var = np.mean(np.square(x), axis=-1, keepdims=True)
    return (x / np.sqrt(var + EPS)) * w


def kernel(x, ln1_w, ln2_w, w_q, w_k, w_v, w_o, attn_gate,
           router, w_up_gate, w_down_moe, w_gate_sh, w_up_sh, w_down_sh):
    x = np.asarray(x, np.float32)
    core_ids = list(range(N_CORES))

    # ---- host prep for launch A
    x_flat = x.reshape(T, D)
    xn = _rms_norm(x_flat, np.asarray(ln1_w, np.float32))
    xT8 = _fp8(xn.T)

    half = ROT_DIM // 2
    inv_freq = 1.0 / ROPE_THETA ** (np.arange(half, dtype=np.float32) / half)
    ang = np.arange(S, dtype=np.float32)[:, None] * inv_freq[None, :]
    cos3 = _bf16(np.tile(np.cos(ang), (1, 3)))
    sin3 = _bf16(np.tile(np.sin(ang), (1, 3)))
    # additive causal mask for the diagonal 128-block: invalid (q < kv) -> -1e4
    maskadd = _bf16(np.where(np.arange(P)[None, :] >= np.arange(P)[:, None],
                             0.0, -1e4))
    gate_full = 2.0 / (1.0 + np.exp(-(xn[:, :G] @ np.asarray(attn_gate, np.float32))))

    w_q = np.asarray(w_q, np.float32)
    w_k = np.asarray(w_k, np.float32)
    w_v = np.asarray(w_v, np.float32)

    if "attn" not in _cache:
        _cache["attn"] = _build_attn()
    ncA = _cache["attn"]

    in_maps = []
    for c in core_ids:
        h0, kv = 2 * c, c // 2
        wpack = np.concatenate(
            [w_q[:, h0 * HD:(h0 + 2) * HD],
             w_k[:, kv * HD:(kv + 1) * HD],
             w_v[:, kv * HD:(kv + 1) * HD]], axis=1)
        in_maps.append(dict(
            xT8=xT8, wp8=_fp8(wpack * WSC),
            cos3=cos3, sin3=sin3,
            gate=np.ascontiguousarray(gate_full[:, h0:h0 + 2]).astype(np.float32),
            maskadd=maskadd))

    resA = run_bass_kernel_spmd(ncA, in_maps, core_ids)

    # assemble [NH*HD, T] fp8 transpose of the per-head attention (still x16)
    attnT8 = np.empty((NH * HD, T), ml_dtypes.float8_e4m3)
    for c in core_ids:
        p = resA.results[c]["po"]        # [T, 2, 64] fp8
        attnT8[2 * c * HD:(2 * c + 2) * HD, :] = \
            p.reshape(T, 2 * HD).T

    # ---- launch B1: w_o per token shard
    if "wo" not in _cache:
        _cache["wo"] = _build_wo()
    ncW = _cache["wo"]
    wo8 = _fp8(np.asarray(w_o, np.float32) * WSC)
    in_maps_w = [dict(aT8=np.ascontiguousarray(attnT8[:, c * TSH:(c + 1) * TSH]),
                      wo8=wo8) for c in core_ids]
    resW = run_bass_kernel_spmd(ncW, in_maps_w, core_ids)

    attn_wo = np.concatenate(
        [resW.results[c]["wout"].astype(np.float32) for c in core_ids],
        axis=0) / (POSC * WSC)
    h = x_flat + attn_wo

    # ---- host routing + dispatch (exact f32)
    hn = _rms_norm(h, np.asarray(ln2_w, np.float32))
    logits = (hn @ np.asarray(router, np.float32)).astype(np.float32)
    logits -= logits.max(-1, keepdims=True)
    pe = np.exp(logits)
    probs = pe / pe.sum(-1, keepdims=True)
    order = np.argsort(-probs, axis=-1, kind="stable")
    sel = order[:, :K]
    wsel = np.take_along_axis(probs, sel, -1)
    wsel = wsel / wsel.sum(-1, keepdims=True)

    idx_e, cw_e = [], []
    for e in range(E):
        hit = (sel == e)
        tok = np.nonzero(hit.any(-1))[0]
        w = (wsel * hit).sum(-1)[tok]
        idx_e.append(tok)
        cw_e.append(w.astype(np.float32))
    maxc = max(len(t) for t in idx_e)
    C = max(256, ((maxc + 255) // 256) * 256)

    if ("moe", C) not in _cache:
        _cache[("moe", C)] = _build_moe(C)
    ncB = _cache[("moe", C)]

    hnT8 = _fp8(hn.T)                    # one quantization, shared by both uses
    w_up_gate = np.asarray(w_up_gate, np.float32)
    w_down_moe = np.asarray(w_down_moe, np.float32)
    wsh_full = _fp8(np.concatenate(
        [np.asarray(w_gate_sh, np.float32),
         np.asarray(w_up_sh, np.float32)], axis=1) * WSC)
    wdsh_full = _fp8(np.asarray(w_down_sh, np.float32) * WSC)

    in_maps_b = []
    for e in range(E):
        tok = idx_e[e]
        xe = np.zeros((D, C), ml_dtypes.float8_e4m3)
        xe[:, :len(tok)] = hnT8[:, tok]
        in_maps_b.append(dict(
            xeT=xe, wug=_fp8(w_up_gate[e] * WSC), wdn=_fp8(w_down_moe[e] * WSC),
            hnT=np.ascontiguousarray(hnT8[:, e * TSH:(e + 1) * TSH]),
            wsh=wsh_full, wdsh=wdsh_full))

    resB = run_bass_kernel_spmd(ncB, in_maps_b, core_ids)

    out = h.copy()
    dsc = 1.0 / (ASC * WSC)
    for e in range(E):
        tok = idx_e[e]
        ye = resB.results[e]["ye"].astype(np.float32)[:len(tok)]
        out[tok] += (cw_e[e] * dsc)[:, None] * ye
        out[e * TSH:(e + 1) * TSH] += dsc * resB.results[e]["ysh"].astype(np.float32)

    return out.reshape(B, S, D).astype(np.float32)
